# revision 35
# baseline (speedup 1.0000x reference)
"""Depthwise 4x4 blur (upfirdn2d pad=(2,1)) on TRN2, 8 NeuronCores.

Math: out[h,w] = sum_{i,j} Kf[i,j] * x[h+i-2, w+j-2]   (Kf = flipped 2D kernel,
out-of-range terms = zero padding). With banded H-blur matrices
A_j[h, h+i-2] = Kf[i, j] (rows clipped to [0,128) = H zero padding) and
W-shifted slices XS_j = Xpad[:, j:j+W]:

    OUT = sum_j A_j @ XS_j

Kf's columns are pairwise equal (A_0 == A_3, A_1 == A_2 for [1,3,3,1] taps),
so the j=0 and j=3 terms collapse through one vector-engine add:

    OUT = A_0 @ t1 + A_1 @ XS_1 + A_1 @ XS_2,    t1 = XS_0 + XS_3

i.e. THREE TensorE matmuls per image (384 streamed cols) instead of four,
with the t1 add split DVE/GPSIMD ~11:5 (their measured 1x rates; the 2x_1P
DVE mode cannot engage because the shift deltas are odd -> 2B-misaligned).
The t1 matmul phase runs last in each set so the PE never waits on the add.

Precision: rel-err budget is 2e-2; fp16 end-to-end costs ~5e-4 (input RNE
2^-11, weights {1,3,9}/64 exact in fp16, PE exact for <=11-bit mantissas,
f32 PSUM accumulate, fp16 output RNE). This halves HBM traffic (the
memory-regime bottleneck) vs f32.

Layouts: host packs per-core input as [H, C*WP] fp16 (image index varies
along the free dim, stride-131 rows [0, 0, x0..x127, 0] bake in the W-pad)
so every supertile DMA is one contiguous multi-KB chunk per partition;
output returns as [H, C*W] fp16, untransposed on host. Input DMAs ride the
SP HWDGE ring, output DMAs the ACT ring. PSUM: 4-image groups (one bank),
4 groups per stationary load, 8-bank double buffering.
Sharding: batch dim (8 batches -> 8 cores), 256 images of 128x128 per core.

A post-build BIR pass (_dedupe_ldweights) drops InstLdweights that reload
the stationary already resident in the PE array (the Bacc path emits one
per matmul; only adjacent-identical loads are removed, with their semaphore
edges migrated to the following matmul).

Measured on HW: ~64-66 us (vs 139.9 us f32 hi/lo baseline); PE wall ~48 us,
DMA ~17 MB at ~330 GB/s effective, ~14 us fixed lead-in/epilogue.
"""

import numpy as np
from contextlib import ExitStack

import concourse.bacc as bacc
import concourse.tile as tile
import concourse.mybir as mybir
from concourse.bass_utils import run_bass_kernel_spmd

N_CORES = 8
B, C, H, W = 8, 256, 128, 128
WP = W + 3         # padded image stride: [0, 0, x0..x127, 0]
GROUP = 4          # images per PSUM bank (4*128 = 512 f32 = one bank)
SET = 4            # PSUM groups per stationary load
SUPER = 16         # images per input DMA
MODE = "fp16"

F32 = mybir.dt.float32
F16 = mybir.dt.float16
ADD = mybir.AluOpType.add


def _body(ctx, tc, o_ap, x_ap, w_ap, ramp=True):
    nc = tc.nc
    wpool = ctx.enter_context(tc.tile_pool(name="wts", bufs=1))
    xpool = ctx.enter_context(tc.tile_pool(name="xin", bufs=6))
    tpool = ctx.enter_context(tc.tile_pool(name="t1", bufs=4))
    opool = ctx.enter_context(tc.tile_pool(name="oup", bufs=5))
    ppool = ctx.enter_context(tc.tile_pool(name="ps", bufs=8, space="PSUM"))

    # weights [H, 2*H] fp16: blocks A_0^T (shared taps j=0,3) and A_1^T
    # (shared taps j=1,2); on the ACT ring so data tiles lead the SP ring
    wt = wpool.tile([H, 2 * H], F16)
    nc.scalar.dma_start(wt[:], w_ap)

    # ramp-up supertile sizes: small leading tiles prime the
    # DMA->add->matmul->copy->DMA pipeline faster (no ramp-down: the last
    # supertile drains while the epilogue barrier runs anyway)
    if ramp:
        sizes = [2, 2, 4, 8] + [SUPER] * ((C - 16) // SUPER)
    else:
        sizes = [SUPER] * (C // SUPER)
    assert sum(sizes) == C
    c0 = 0
    for size_i, sz in enumerate(sizes):
        last_super = size_i == len(sizes) - 1
        xt = xpool.tile([H, sz * WP], F16, tag="xt")
        if sz <= 8:
            # small ramp tiles: split across both HWDGE rings so descriptor
            # generation for the two halves runs in parallel
            hh = sz // 2
            nc.sync.dma_start(xt[:, : hh * WP], x_ap[:, c0 * WP : (c0 + hh) * WP])
            nc.scalar.dma_start(
                xt[:, hh * WP : sz * WP], x_ap[:, (c0 + hh) * WP : (c0 + sz) * WP]
            )
        else:
            nc.sync.dma_start(xt[:], x_ap[:, c0 * WP : (c0 + sz) * WP])
        xt3 = xt[:].rearrange("h (c w) -> h c w", c=sz)
        # t1 = XS_0 + XS_3 (taps with equal kernel columns); halves on DVE
        # and GPSIMD so neither vector engine becomes the bottleneck
        t1 = tpool.tile([H, sz * W], F16, tag="t1")
        t13 = t1[:].rearrange("h (c w) -> h c w", c=sz)
        # measured rates: DVE ~1.5 elem/ns, GPSIMD ~0.6 -> split ~11:5
        hh2 = max(1, (sz * 11) // 16)
        nc.vector.tensor_tensor(
            t13[:, :hh2], xt3[:, :hh2, 0:W], xt3[:, :hh2, 3 : 3 + W], ADD
        )
        if hh2 < sz:
            nc.gpsimd.tensor_tensor(
                t13[:, hh2:], xt3[:, hh2:, 0:W], xt3[:, hh2:, 3 : 3 + W], ADD
            )
        ot = opool.tile([H, sz * W], F16, tag="ot")
        # PSUM groups of GROUP images; SET groups share each stationary load
        groups = [
            (i * GROUP, min(GROUP, sz - i * GROUP))
            for i in range((sz + GROUP - 1) // GROUP)
        ]
        sets = [tuple(groups[i : i + SET]) for i in range(0, len(groups), SET)]
        for gs in sets:
            i0, iend = gs[0][0], gs[-1][0] + gs[-1][1]
            pts = []
            for g in gs:
                pt = ppool.tile([H, g[1] * W], F32, tag="pt")
                pts.append(pt)
            # per group: pt = A0 @ t1 + A1 @ XS_1 + A1 @ XS_2 (3 matmuls,
            # 2 stationaries). The t1 phase runs LAST in every set so the
            # PE never stalls on the vector-engine add of the newest
            # supertile (worth the non-deduped A1 reload each set).
            phases = [(1, xt3, 1), (1, xt3, 2), (0, t13, None)]
            np_h = len(phases)
            for kp, (t, src3, j) in enumerate(phases):
                lhsT = wt[:, t * H : (t + 1) * H]
                for (gi, gc), pt in zip(gs, pts):
                    if j is None:
                        rhs = src3[:, gi : gi + gc, :]
                    else:
                        rhs = src3[:, gi : gi + gc, j : j + W]
                    nc.tensor.matmul(
                        pt[:], lhsT, rhs, start=(kp == 0), stop=(kp == np_h - 1)
                    )
            for (gi, gc), pt in zip(gs, pts):
                # PSUM->SBUF fp16 downcast copies all on ScalarE (GPSIMD
                # cannot read PSUM; DVE is saturated by the t1 adds)
                nc.scalar.copy(ot[:, gi * W : (gi + gc) * W], pt[:])
                if last_super:
                    # final supertile: drain per group, alternating rings,
                    # so the kernel tail is not one bulk DMA flush
                    eng = nc.sync if gi % 8 else nc.scalar
                    eng.dma_start(
                        o_ap[:, (c0 + gi) * W : (c0 + gi + gc) * W],
                        ot[:, gi * W : (gi + gc) * W],
                    )
            if not last_super:
                # per-set output DMA on the ACT ring (inputs own the SP ring)
                nc.scalar.dma_start(
                    o_ap[:, (c0 + i0) * W : (c0 + iend) * W],
                    ot[:, i0 * W : iend * W],
                )
        c0 += sz


def _dedupe_ldweights(nc):
    """Drop InstLdweights that reload the stationary already in the PE array.

    Matmuls in the Bacc path never self-load (ldweights=False, one companion
    InstLdweights each), so an LDW whose access pattern equals the previous
    LDW's is a no-op. Only waitless/updateless dups are removed so no
    tile-framework semaphore edges are lost.
    """
    removed = 0
    for f in nc.m.functions:
        last_key = None
        for blk in f.blocks:
            insts = blk.instructions
            i = 0
            while i < len(insts):
                inst = insts[i]
                tn = type(inst).__name__
                if tn == "InstLdweights":
                    key = repr(inst.ins[0])
                    if key == last_key and i + 1 < len(insts):
                        si = inst.sync_info
                        if si is not None and (si.on_wait or si.on_update):
                            # merge the dup's semaphore edges into the next
                            # instruction (the matmul that follows it on the
                            # PE queue) so no tile-framework sync is lost
                            nxt = insts[i + 1]
                            ns = nxt.sync_info
                            if ns is None:
                                nxt.sync_info = mybir.SyncInfo(
                                    on_wait=list(si.on_wait),
                                    on_update=list(si.on_update),
                                )
                            else:
                                nxt.sync_info = mybir.SyncInfo(
                                    on_wait=list(ns.on_wait) + list(si.on_wait),
                                    on_update=list(ns.on_update)
                                    + list(si.on_update),
                                )
                        del insts[i]
                        removed += 1
                        continue
                    last_key = key
                elif tn == "InstMatmult":
                    assert inst.ldweights is False
                i += 1
    return removed


def build_module(ramp=True):
    nc = bacc.Bacc(
        "TRN2", target_bir_lowering=False, debug=False, num_devices=N_CORES
    )
    x_ap = nc.dram_tensor("x", [H, C * WP], F16, kind="ExternalInput").ap()
    w_ap = nc.dram_tensor("wts", [H, 2 * H], F16, kind="ExternalInput").ap()
    o_ap = nc.dram_tensor("out", [H, C * W], F16, kind="ExternalOutput").ap()
    with tile.TileContext(nc) as tc:
        with ExitStack() as ctx:
            _body(ctx, tc, o_ap, x_ap, w_ap, ramp=ramp)
    _dedupe_ldweights(nc)
    nc.compile()
    return nc


def band_mats(k2d):
    """WT[j] = A_j^T where A_j[h, h+i-2] = Kf[i, j] (rows clipped to [0,128))."""
    kf = np.asarray(k2d, np.float32)[::-1, ::-1]
    wts = np.zeros((4, H, H), np.float32)
    for j in range(4):
        for i in range(4):
            d = i - 2  # diagonal offset m - h
            h0, h1 = max(0, -d), min(H, H - d)
            idx = np.arange(h0, h1)
            wts[j, idx + d, idx] = kf[i, j]
    return wts


def pack_x(x_core):
    """[C,H,W] f32 -> [H, C*WP] fp16 with zero cols at 0,1 and WP-1."""
    xh = np.zeros((H, x_core.shape[0], WP), np.float16)
    xh[:, :, 2 : 2 + W] = x_core.transpose(1, 0, 2)
    return xh.reshape(H, -1)


_module_cache = {}


def _get_module(mode=MODE):
    if mode not in _module_cache:
        _module_cache[mode] = build_module()
    return _module_cache[mode]


def kernel(x, kernel, _trace=False, _trace_kwargs=None, _mode=None):
    x = np.asarray(x, np.float32)
    assert x.shape == (B, C, H, W), x.shape
    bm = band_mats(kernel)
    kf = np.asarray(kernel, np.float32)[::-1, ::-1]
    assert np.allclose(kf[:, 0], kf[:, 3]) and np.allclose(kf[:, 1], kf[:, 2]), (
        "W taps not pairwise symmetric"
    )
    wts = (
        np.stack([bm[0], bm[1]])
        .transpose(1, 0, 2)
        .reshape(H, 2 * H)
        .astype(np.float16)
    )
    nc = _get_module(_mode or MODE)
    in_maps = [{"x": pack_x(x[i]), "wts": wts} for i in range(N_CORES)]
    res = run_bass_kernel_spmd(
        nc, in_maps, list(range(N_CORES)), trace=_trace, **(_trace_kwargs or {})
    )
    out = np.stack(
        [
            res.results[i]["out"]
            .reshape(H, C, W)
            .transpose(1, 0, 2)
            .astype(np.float32)
            for i in range(N_CORES)
        ],
        axis=0,
    )
    if _trace:
        return out, res
    return out
